# revision 47
# baseline (speedup 1.0000x reference)
"""Trainium2 Bass kernel for nn_DMlp_46823733461564 (dense_mlp).

Computes: token-grid 3x3 masked-neighborhood gather (pixel-shuffle +
reflection-pad + masked unfold, algebraically reduced to a channel-
permuted shifted gather) followed by fc1(1600->1024) + exact GELU +
fc2(1024->576).

Sharding: data-parallel over (batch, image-half) -> 8 cores, 8192 tokens
each; fc weights replicated. The gather runs on-device as DMAs from a
host-prepared reflection-extended channel-major image; matmuls run in
bf16 on the PE (fp32 PSUM accumulation), ~3e-3 relative error.

Performance notes (from NTFF traces):
- Each DMA_DIRECT2D costs ~600ns of *issue* time on its engine queue, so
  DMA count is minimized: host packs w1 into 4 mp-major slabs, w2 into
  one slab, and the gather source is partition-major/row-outer so each
  feat tile is ONE DMA with 3.3-6.6KB contiguous runs.
- Only the sync (SP) and scalar (Activation) queues are hardware-DGE;
  gpsimd's queue is software-DGE with ~4us startup lag -- unused.
  Weights are need-time ordered: mp0 behind tile-0's gather on sync,
  mp1 right after, mp2/mp3/w2/b2 on scalar. The scalar queue then stays
  free for GELUs (an early ACT stall gates the PE clock to 4/8 and
  costs ~2x for tens of us; the Gelu table is pre-warmed for the same
  reason).
- K=1600 is padded to 13x128 with a zero half-chunk (zeros streamed
  from DRAM): every fc1 matmul is a full (128,128)-tile op. The PE
  row-group (64,128) tile mode costs ~300ns per isolated mode switch,
  more than the half-empty stream it replaces. (A fully band-paired fc1
  that would recover the half-chunk hits a toolchain bug: band-tiled
  matmuls fed by engine-written SBUF or in this kernel's full shape die
  with a runtime internal error, though they pass CoreSim.)
- Small first/last tiles (256 tokens) start compute earlier and shorten
  the serial fc2 tail; warmup matmuls cover the boot->first-data window
  so the HAM clock gate reaches 8/8 before real work.
"""
import os
import sys

import numpy as np

_TRN_REPO = "/opt/trn_rl_repo"
if _TRN_REPO not in sys.path:
    sys.path.insert(0, _TRN_REPO)

B, HIMG, WIMG = 4, 128, 128
C = 64
L = 576           # C * 9
NTOK = HIMG * WIMG
HID = 1024
OUTF = 576
INF = 1600        # C * 25
N_CORES = 8
ROWS_PER_CORE = HIMG // 2          # 64 token rows
TOK_PER_CORE = ROWS_PER_CORE * WIMG  # 8192
KC = 13                            # K-chunks of 128 (last half zero-padded)
NMP = HID // 256                   # 4 m-pairs

_MASK = np.array([
    [1, 0, 0, 1, 0, 0, 1],
    [0, 1, 0, 1, 0, 1, 0],
    [0, 0, 1, 1, 1, 0, 0],
    [1, 1, 1, 1, 1, 1, 1],
    [0, 0, 1, 1, 1, 0, 0],
    [0, 1, 0, 1, 0, 1, 0],
    [1, 0, 0, 1, 0, 0, 1]], dtype=bool)
MASK_POS = [(i, j) for i in range(7) for j in range(7) if _MASK[i, j]]


def _dmap(d):
    if d <= 1:
        return -1, d + 1
    if d <= 4:
        return 0, d - 2
    return 1, d - 5


KPOS = []
for (_di, _dj) in MASK_POS:
    _dh, _r1 = _dmap(_di)
    _dw, _r2 = _dmap(_dj)
    KPOS.append((_dh, _dw, _r1 * 3 + _r2))


def _swap_map(a, b, which):
    ch = np.arange(L)
    c, rem = ch // 9, ch % 9
    r1, r2 = rem // 3, rem % 3
    r = r1 if which == 0 else r2
    rs = np.where(r == a, b, np.where(r == b, a, r))
    if which == 0:
        return c * 9 + rs * 3 + r2
    return c * 9 + r1 * 3 + rs


def _build_xe(x):
    """x: (B, NTOK, L) -> xe: (B, L, HIMG+2, WIMG+2) reflection-extended,
    channel-permuted borders."""
    xt = np.ascontiguousarray(x.transpose(0, 2, 1)).reshape(B, L, HIMG, WIMG)
    xe = np.empty((B, L, HIMG + 2, WIMG + 2), dtype=np.float32)
    xe[:, :, 1:-1, 1:-1] = xt
    xe[:, :, 0, 1:-1] = xt[:, _swap_map(1, 2, 0), 0, :]
    xe[:, :, -1, 1:-1] = xt[:, _swap_map(0, 1, 0), -1, :]
    xe[:, :, :, 0] = np.take(xe[:, :, :, 1], _swap_map(1, 2, 1), axis=1)
    xe[:, :, :, -1] = np.take(xe[:, :, :, -2], _swap_map(0, 1, 1), axis=1)
    return xe


_NC_CACHE = {}

MODE = os.environ.get("KERNEL_DTYPE", "bf16")  # "f32r" | "bf16"

_WS_COUNTER = [0]


def _split_waits(nc, limit=1):
    """walrus in this toolchain accepts only ONE sync wait per instruction;
    move excess waits onto same-engine NoOps inserted just before (engine
    program order makes this equivalent)."""
    import concourse.mybir as mybir

    def noop(engine, waits):
        _WS_COUNTER[0] += 1
        return mybir.InstNoOp(
            name=f"WS-{_WS_COUNTER[0]}",
            sync_info=mybir.SyncInfo(on_wait=list(waits), on_update=[]),
            bass_nofuse=True,
            engine=engine,
        )

    for fn in nc.m.functions:
        for blk in fn.blocks:
            new_insts = []
            for inst in blk.instructions:
                si = getattr(inst, "sync_info", None)
                waits = list(si.on_wait) if si and si.on_wait else []
                if len(waits) > limit:
                    excess = waits[: len(waits) - limit]
                    si.on_wait = waits[len(waits) - limit:]
                    while excess:
                        new_insts.append(noop(inst.engine, excess[:limit]))
                        excess = excess[limit:]
                new_insts.append(inst)
            blk.instructions = new_insts


def _build_bass():
    if "nc" in _NC_CACHE:
        return _NC_CACHE["nc"]
    import concourse.bass as bass
    import concourse.mybir as mybir
    from concourse.tile import TileContext

    f32 = mybir.dt.float32
    mm_dt = mybir.dt.float32r if MODE == "f32r" else mybir.dt.bfloat16
    AF = mybir.ActivationFunctionType
    Alu = mybir.AluOpType

    nc = bass.Bass("TRN2", target_bir_lowering=False, debug=False)
    # row-outer layout: a tile gather is one contiguous slab per partition
    # (nrows*KC*128 elements), so the DMA moves 3.3-6.6KB runs at full rate
    xq = nc.dram_tensor("xq", (128, ROWS_PER_CORE, KC, WIMG), mm_dt,
                        kind="ExternalInput")
    w1q = nc.dram_tensor("w1q", (NMP, 128, KC * 256), mm_dt,
                         kind="ExternalInput")
    w2q = nc.dram_tensor("w2q", (128, (HID // 128) * OUTF), mm_dt,
                         kind="ExternalInput")
    b1rs = nc.dram_tensor("b1rs", (128, HID // 128), f32, kind="ExternalInput")
    b2bc = nc.dram_tensor("b2bc", (128, OUTF), f32, kind="ExternalInput")
    out = nc.dram_tensor("out", (TOK_PER_CORE, OUTF), f32,
                         kind="ExternalOutput")

    with TileContext(nc) as tc:
        with (
            tc.tile_pool(name="wpool", bufs=1) as wpool,
            tc.tile_pool(name="fpool", bufs=3) as fpool,
            tc.tile_pool(name="hpool", bufs=2) as hpool,
            tc.tile_pool(name="opool", bufs=3) as opool,
            tc.tile_pool(name="ps1", bufs=2, space="PSUM") as ps1,
            tc.tile_pool(name="ps2", bufs=2, space="PSUM") as ps2,
        ):
            # --- PE warmup: dependency-free matmuls fill the initial DMA
            # wait and push the HAM clock gate to 8/8 before real work ---
            n_warm = int(os.environ.get("KERNEL_WARMUP", "45"))
            warm_n = int(os.environ.get("KERNEL_WARMUP_N", "128"))
            n_pad = int(os.environ.get("KERNEL_WARMPAD", "2"))
            warm = None
            wps = None
            if n_warm or n_pad:
                warm = wpool.tile([128, 512], mm_dt, tag="warm")
                nc.vector.memset(warm[:, :], 0.0)
                # pre-warm the scalar engine's Gelu table (ACT_TABLE_LOAD
                # is ~1.3us; do it now, not before the first real GELU)
                dummy = wpool.tile([128, 16], mm_dt, tag="dummy")
                nc.scalar.activation(dummy[:, :], warm[:, 0:16], AF.Gelu,
                                     bias=0.0, scale=1.0)
                wps = ps2.tile([128, 512], f32, tag="poa")
                for _ in range(n_warm):
                    nc.tensor.matmul(wps[:, 0:warm_n], warm[:, 0:128],
                                     warm[:, 0:warm_n], start=True, stop=True)

            # --- replicated weights, ordered by need time across the two
            # HW DGE queues (gpsimd's queue is software-DGE: ~4us startup
            # lag and slow, so it gets nothing). sync: w1-mp0 then tile-0
            # gather then w1-mp1 (emitted inside the loop); scalar: b1,
            # mp2, mp3, w2, b2, then it's free for GELUs. ---
            b1t = wpool.tile([128, HID // 128], f32, tag="b1")
            nc.scalar.dma_start(out=b1t[:, :], in_=b1rs[:, :])
            w1sb = []
            for mp in range(NMP):
                t = wpool.tile([128, KC * 256], mm_dt, tag=f"w1_{mp}")
                w1sb.append(t)
            nc.sync.dma_start(out=w1sb[0][:, :], in_=w1q[0, :, :])
            nc.scalar.dma_start(out=w1sb[2][:, :], in_=w1q[2, :, :])
            nc.scalar.dma_start(out=w1sb[3][:, :], in_=w1q[3, :, :])
            w2all = wpool.tile([128, (HID // 128) * OUTF], mm_dt, tag="w2")
            nc.scalar.dma_start(out=w2all[:, :], in_=w2q[:, :])
            b2t = wpool.tile([128, OUTF], f32, tag="b2")
            nc.scalar.dma_start(out=b2t[:, :], in_=b2bc[:, :])

            def emit_fc2(hts, r0, tt, out_eng=None):
                # --- fc2: out[tok, :] = h.T @ w2 + b2 ---
                # N split 288+288; each [128, 288] psum is one bank.
                NH = OUTF // 2
                ns = tt // 128
                for s in range(ns):
                    poa = ps2.tile([128, NH], f32, tag="poa")
                    pob = ps2.tile([128, NH], f32, tag="pob")
                    for j in range(HID // 128):
                        nc.tensor.matmul(
                            poa[:, :],
                            hts[j][:, s * 128: (s + 1) * 128],
                            w2all[:, j * OUTF: j * OUTF + NH],
                            start=(j == 0), stop=(j == HID // 128 - 1),
                        )
                        nc.tensor.matmul(
                            pob[:, :],
                            hts[j][:, s * 128: (s + 1) * 128],
                            w2all[:, j * OUTF + NH: (j + 1) * OUTF],
                            start=(j == 0), stop=(j == HID // 128 - 1),
                        )
                    ot = opool.tile([128, OUTF], f32, tag="o")
                    nc.vector.tensor_tensor(
                        out=ot[:, 0:NH], in0=poa[:, :],
                        in1=b2t[:, 0:NH], op=Alu.add)
                    nc.vector.tensor_tensor(
                        out=ot[:, NH:OUTF], in0=pob[:, :],
                        in1=b2t[:, NH:OUTF], op=Alu.add)
                    tok0 = (r0 * WIMG) + s * 128
                    (out_eng or nc.sync).dma_start(
                        out=out[tok0: tok0 + 128, :], in_=ot[:, :])

            # tile list: small head tiles (compute starts after ~0.4MB of
            # gather instead of 1.7MB) and small tail tiles (short serial
            # fc2 tail after the final fc1)
            tiles = [(0, 2), (2, 2)]
            tiles += [(4 + 4 * i, 4) for i in range((ROWS_PER_CORE - 8) // 4)]
            tiles += [(ROWS_PER_CORE - 4, 2), (ROWS_PER_CORE - 2, 2)]

            prev = None  # (hts, r0, tt) of the previous tile: fc2 runs one
            # tile behind fc1 so the PE never waits on the GELU latency
            for t_i, (r0, nrows) in enumerate(tiles):
                tt = nrows * WIMG
                # --- gather featT tile: ONE DMA; host laid the source out
                # partition-major/row-outer with the (dh, dw, q) shifts
                # pre-applied, so the slab is contiguous per partition ---
                fall = fpool.tile([128, KC * tt], mm_dt, tag="fall")
                dst = fall[:, :].rearrange("p (r j w) -> p r j w",
                                           j=KC, r=nrows)
                nc.sync.dma_start(out=dst, in_=xq[:, r0: r0 + nrows, :, :])
                if t_i == 0:
                    # w1 mp1 lands on sync right behind tile-0's gather
                    nc.sync.dma_start(out=w1sb[1][:, :], in_=w1q[1, :, :])
                fv = fall[:, :].rearrange("p (r j w) -> p j r w",
                                          j=KC, r=nrows)
                fts = [fv[:, j] for j in range(KC)]
                # --- fc1 + GELU: h[m] = gelu(w1.T @ featT + b1); m-pairs
                # interleave two psum chains so LDWEIGHTS stays hidden ---
                hts = []
                for mp in range(NMP):
                    m0, m1 = 2 * mp, 2 * mp + 1
                    psa = ps1.tile([128, tt], f32, tag="psa")
                    psb = ps1.tile([128, tt], f32, tag="psb")
                    for j in range(KC):
                        nc.tensor.matmul(
                            psa[:, :],
                            w1sb[mp][:, j * 256: j * 256 + 128],
                            fts[j], start=(j == 0), stop=(j == KC - 1))
                        nc.tensor.matmul(
                            psb[:, :],
                            w1sb[mp][:, j * 256 + 128: (j + 1) * 256],
                            fts[j], start=(j == 0), stop=(j == KC - 1))
                    for m, pst in ((m0, psa), (m1, psb)):
                        ht = hpool.tile([128, tt], mm_dt, tag=f"h{m}")
                        nc.scalar.activation(ht[:, :], pst[:, :], AF.Gelu,
                                             bias=b1t[:, m: m + 1], scale=1.0)
                        hts.append(ht)
                    if t_i < 2 and n_pad:
                        # ramp insurance: keep the PE fed through any DMA
                        # jitter so the HAM gate never drops below 8/8
                        for _ in range(n_pad):
                            nc.tensor.matmul(wps[:, 0:warm_n], warm[:, 0:128],
                                             warm[:, 0:warm_n],
                                             start=True, stop=True)
                if prev is not None:
                    emit_fc2(*prev)
                prev = (hts, r0, tt)
            emit_fc2(*prev)

    _split_waits(nc)
    _NC_CACHE["nc"] = nc
    return nc


def _host_prep(x, w1, b1, w2, b2):
    x = np.ascontiguousarray(np.asarray(x, dtype=np.float32))
    w1 = np.asarray(w1, dtype=np.float32)
    b1 = np.asarray(b1, dtype=np.float32)
    w2 = np.asarray(w2, dtype=np.float32)
    b2 = np.asarray(b2, dtype=np.float32)

    xe = _build_xe(x)
    w1t = np.ascontiguousarray(w1.T)  # (1600, 1024) rows c*25+k
    w1p = np.ascontiguousarray(
        w1t.reshape(C, 25, HID).transpose(1, 0, 2).reshape(INF, HID))
    w2t = np.ascontiguousarray(w2.T)  # (1024, 576)
    b1rs = np.ascontiguousarray(b1.reshape(HID // 128, 128).T)
    b2bc = np.ascontiguousarray(np.broadcast_to(b2, (128, OUTF)))

    if MODE == "bf16":
        import ml_dtypes
        xe = xe.astype(ml_dtypes.bfloat16)
        w1p = w1p.astype(ml_dtypes.bfloat16)
        w2t = w2t.astype(ml_dtypes.bfloat16)

    # w1 packed as 4 mp-major slabs [128, KC*256]; chunk 12 rows 64:128
    # duplicate rows 0:64 (finite filler against the zero-padded feat)
    w1q = np.empty((NMP, 128, KC * 256), dtype=w1p.dtype)
    for j in range(KC):
        kr = min(128, INF - j * 128)
        rows = w1p[j * 128: j * 128 + kr]
        if kr < 128:
            rows = np.concatenate([rows, rows], axis=0)
        for mp in range(NMP):
            w1q[mp, :, j * 256: (j + 1) * 256] = rows[
                :, mp * 256: (mp + 1) * 256]
    # w2 packed as one slab [128, 8*576]
    w2qa = np.empty((128, (HID // 128) * OUTF), dtype=w2t.dtype)
    for j in range(HID // 128):
        w2qa[:, j * OUTF: (j + 1) * OUTF] = w2t[j * 128: (j + 1) * 128, :]

    in_maps = []
    for cid in range(N_CORES):
        b, half = cid // 2, cid % 2
        h0 = half * ROWS_PER_CORE
        xpair = np.zeros((KC, 128, ROWS_PER_CORE, WIMG), dtype=xe.dtype)
        for j in range(KC):
            for p in range(2):
                k = 2 * j + p
                if k > 24:
                    continue  # zero pad: K 1600 -> 1664
                dh, dw, q = KPOS[k]
                xpair[j, p * 64: (p + 1) * 64] = xe[
                    b, q::9,
                    1 + h0 + dh: 1 + h0 + dh + ROWS_PER_CORE,
                    1 + dw: 1 + dw + WIMG]
        xqa = np.ascontiguousarray(xpair.transpose(1, 2, 0, 3))
        in_maps.append({
            "xq": xqa, "w1q": w1q, "w2q": w2qa, "b1rs": b1rs, "b2bc": b2bc,
        })
    return in_maps


def _assemble(results):
    out = np.empty((B, NTOK, OUTF), dtype=np.float32)
    for cid in range(N_CORES):
        b, half = cid // 2, cid % 2
        t0 = half * TOK_PER_CORE
        out[b, t0: t0 + TOK_PER_CORE, :] = results[cid]["out"]
    return out


def kernel(x, w1, b1, w2, b2, image_h, image_w):
    in_maps = _host_prep(x, w1, b1, w2, b2)
    nc = _build_bass()
    from concourse.bass_utils import run_bass_kernel_spmd
    res = run_bass_kernel_spmd(nc, in_maps, list(range(N_CORES)))
    return _assemble(res.results)
